# revision 8
# baseline (speedup 1.0000x reference)
"""Single-head attention (B=4, T=4096, D=1024, H=64, fp32 in/out) on 8 TRN2
NeuronCores.

Sharding: one core per (batch, T-half) pair -> 8 shards, no collectives.
Host pre-transposes/pre-casts/pre-packs inputs (zero device-side transposes):
  xt      [8*128, 8*512] bf16  per t-block: [128 part, (d-chunk, 512)] of
                               x[b]^T (query t-blocks first)
  wqt     [128, 8*64]    bf16  Wq^T packed [part, (d-chunk, 64)]
  wkvt    [128, 8*128]   bf16  [Wk^T | Wv^T] packed likewise
  maskt   [4096, 2048]   bf16  mask slice transposed to [s, t]
Each core returns un-normalized [65, 2048] (out'^T rows 0:64, softmax
denominator row 64); the host does the final divide + transpose.

Architecture of the schedule. Act's exp stream is the hard floor (8.4M
elements / 128 lanes / ~0.9 GHz ~= 72 us) and the PE is co-saturated
(ST 128 + PV 128 + proj 96 matmuls at ~215 ns N=512 issue ~= 76 us), so
the kernel must (a) start the exp stream early, (b) never let any
engine's in-order stream stall it, (c) smooth all projection work into
fine grains that fit per-step slack:

  - Two column streams: A = query cols 0:1024 (PV supertiles 0,1),
    B = cols 1024:2048 (2,3). Order A0..A15, (A16,B0)..(A31,B15),
    B16..B31. exp-A0 needs only q0,q1,kv0.
  - Stream A's PV lags 12 steps: steps 0..11 carry no PV matmuls, which
    is exactly the slack that absorbs the kv1-kv4 + q2,q3 projection
    quarters (2 matmuls each, PSUM-gen-local). Stream B's PV lags 3.
    A's last 12 PV chunks drain into the B-only tail steps.
  - Projection quarters accumulate 2 d-chunks in PSUM, a DVE copy moves
    them to SBUF f32, and an add chain (DVE for early blocks, GpSimd --
    which has no other mid-kernel work -- for late blocks) merges into
    bf16 kvT. No PSUM generation outlives its step, so the 2-deep ST
    ping-pong never serializes.
  - DMA: queues are FIFO internally but round-robin against each other,
    so the sync HW queue carries x0,x1 chunk-pipelined (projections
    chase landing chunks), then x2,x3,x4, then per-step mask halves
    (each stream fetches only its own [128,1024] half when it needs
    it); x5-x7 dispatch from inside the loop. Weights ride the scalar
    HW queue in parallel. Deep PT ring (20) + lagged PV decouple mask
    lateness from the exp stream.
  - ST matmuls use full-128-row kvT / zero-padded qT so the whole PE
    array stays active (HAM clock gate 1.2 -> 2.4 GHz); dummy warmup
    matmuls on a memset tile ramp the clock with no DMA dependency.
  - Tail: stream B's PV trail tightens over the last two steps, the
    final PV/copy/store is split per 512-col supertile, epilogue copies
    run on Vector/Act after the last exp; host divides.
"""

import sys

if "/opt/trn_rl_repo" not in sys.path:
    sys.path.insert(0, "/opt/trn_rl_repo")

from contextlib import ExitStack

import numpy as np
import ml_dtypes

import concourse.bass as bass
import concourse.tile as tile
from concourse import bacc, mybir
from concourse.bass_utils import run_bass_kernel_spmd
from concourse.masks import make_identity

F32 = mybir.dt.float32
BF16 = mybir.dt.bfloat16

B, T, D, H = 4, 4096, 1024, 64
NCORES = 8
TQ = T // 2  # query rows per core

BF16NP = ml_dtypes.bfloat16


def build_attention_core(T=T, D=D, H=H, Tq=TQ):
    """Build the per-core Bass graph. Every core runs the same graph."""
    assert D % 128 == 0 and T % 1024 == 0 and Tq % 1024 == 0 and H == 64
    DC = D // 128          # d chunks (8)
    NS = T // 128          # s chunks (32)
    NTB = T // 512         # x t-blocks (8)
    NQB = Tq // 512        # query t-blocks (4)
    LAG_A = 12             # stream A PV lag (frees early steps of PE)
    LAG_B = 3              # stream B PV lag
    scale = 1.0 / float(np.sqrt(D))
    Exp = mybir.ActivationFunctionType.Exp
    ADD = mybir.AluOpType.add

    nc = bacc.Bacc("TRN2", target_bir_lowering=False, debug=False,
                   num_devices=NCORES)
    xT_ext = nc.declare_dram_parameter("xt", [NTB * 128, DC * 512], BF16,
                                       isOutput=False)
    wqT_ext = nc.declare_dram_parameter("wqt", [128, DC * H], BF16,
                                        isOutput=False)
    wkvT_ext = nc.declare_dram_parameter("wkvt", [128, DC * 2 * H], BF16,
                                         isOutput=False)
    maskT_ext = nc.declare_dram_parameter("maskt", [T, Tq], BF16,
                                          isOutput=False)
    out_ext = nc.declare_dram_parameter("out", [H + 1, Tq], BF16,
                                        isOutput=True)

    with tile.TileContext(nc) as tc, ExitStack() as ctx:
        singles = ctx.enter_context(tc.tile_pool(name="singles", bufs=1))
        xin = ctx.enter_context(tc.tile_pool(name="xin", bufs=1))
        mpool = ctx.enter_context(tc.tile_pool(name="mpool", bufs=6))
        ptpool = ctx.enter_context(tc.tile_pool(name="ptpool", bufs=24))
        apool = ctx.enter_context(tc.tile_pool(name="apool", bufs=2))
        opool = ctx.enter_context(tc.tile_pool(name="opool", bufs=1))
        # PSUM: tag "st" [128,2,512] f32 x2 (4 banks) shared by ST tiles,
        # projection quarters, V' transposes and warmup; tag "pv"
        # [128,4,512] f32 x1 (4 banks) = (stream,ts) accumulators.
        psP = ctx.enter_context(tc.tile_pool(name="psP", bufs=2,
                                             space="PSUM"))
        psV = ctx.enter_context(tc.tile_pool(name="psV", bufs=1,
                                             space="PSUM"))

        # ---- persistent SBUF ----
        wqT_sb = singles.tile([128, DC, H], BF16)
        wkvT_sb = singles.tile([128, DC, 2 * H], BF16)
        ident_bf = singles.tile([128, 128], BF16)
        warm_sb = singles.tile([128, 512], BF16)
        kvT_sb = singles.tile([128, T], BF16)   # rows 0:64 kT, 64:128 vT
        qT_sb = singles.tile([128, Tq], BF16)   # rows 64:128 zero
        Vp_sb = singles.tile([128, NS, 128], BF16)  # V' = [V | 1 | 0pad]

        # ---- weights on the scalar HW queue (parallel with x on sync) ----
        nc.scalar.dma_start(
            out=wqT_sb.rearrange("p a b -> p (a b)"), in_=wqT_ext[:, :]
        )
        nc.scalar.dma_start(
            out=wkvT_sb.rearrange("p a b -> p (a b)"), in_=wkvT_ext[:, :]
        )

        # ---- x0, x1 chunk-pipelined, then x2-x4, on the sync HW queue ----
        x_tiles = {}
        for b in range(2):
            x_sb = xin.tile([128, DC, 512], BF16, tag="x", bufs=NTB,
                            name="x_sb")
            for c in range(0, DC, 2):
                nc.sync.dma_start(
                    out=x_sb[:, c : c + 2, :].rearrange("p a b -> p (a b)"),
                    in_=xT_ext[b * 128 : (b + 1) * 128,
                               c * 512 : (c + 2) * 512],
                )
            x_tiles[b] = x_sb
        for b in range(2, 5):
            x_sb = xin.tile([128, DC, 512], BF16, tag="x", bufs=NTB,
                            name="x2_sb")
            nc.sync.dma_start(
                out=x_sb.rearrange("p a b -> p (a b)"),
                in_=xT_ext[b * 128 : (b + 1) * 128, :],
            )
            x_tiles[b] = x_sb

        def x_late(b):
            """Dispatch a late x block from inside the loop (sync queue)."""
            x_sb = xin.tile([128, DC, 512], BF16, tag="x", bufs=NTB,
                            name="xl_sb")
            nc.sync.dma_start(
                out=x_sb.rearrange("p a b -> p (a b)"),
                in_=xT_ext[b * 128 : (b + 1) * 128, :],
            )
            x_tiles[b] = x_sb

        # ---- gpsimd setup (runs while DMAs fly) ----
        nc.gpsimd.memset(warm_sb, 1.0)
        nc.gpsimd.memset(qT_sb[H : 2 * H, :], 0.0)
        make_identity(nc, ident_bf)
        nc.gpsimd.memset(Vp_sb[:, :, H + 1 : 128], 0.0)
        nc.gpsimd.memset(Vp_sb[:, :, H : H + 1], 1.0)

        # ---- PE warmup: ramp the clock before real work (no DMA dep) ----
        for _ in range(3):
            w_ps = psP.tile([128, 2, 512], F32, tag="st", name="w_ps")
            for i in range(2):
                nc.tensor.matmul(w_ps[:, i, :], warm_sb[:, 0:128], warm_sb)

        # ---- prologue: q0 + kv0 interleaved (chase x0), then q1 ----
        qkv_ps = psP.tile([128, 2, 512], F32, tag="st", name="qkv_ps")
        for c in range(DC):
            nc.tensor.matmul(
                qkv_ps[0:H, 0, :], wqT_sb[:, c, :], x_tiles[0][:, c, :],
                start=(c == 0), stop=(c == DC - 1),
            )
            nc.tensor.matmul(
                qkv_ps[:, 1, :], wkvT_sb[:, c, :], x_tiles[0][:, c, :],
                start=(c == 0), stop=(c == DC - 1),
            )
        nc.vector.tensor_copy(qT_sb[0:H, 0:512], qkv_ps[0:H, 0, :])
        nc.vector.tensor_copy(kvT_sb[:, 0:512], qkv_ps[:, 1, :])
        q1_ps = psP.tile([128, 2, 512], F32, tag="st", name="q1_ps")
        for c in range(DC):
            nc.tensor.matmul(
                q1_ps[0:H, 0, :], wqT_sb[:, c, :], x_tiles[1][:, c, :],
                start=(c == 0), stop=(c == DC - 1),
            )
        nc.vector.tensor_copy(qT_sb[0:H, 512:1024], q1_ps[0:H, 0, :])

        # ---- projection quarters (2 d-chunks in PSUM, merged via SBUF) ----
        acc_kv = {}
        acc_q = {}

        def kv_quarter(b, qi, merge_eng):
            """2-chunk kv accumulation for block b; merge via SBUF f32."""
            ps = psP.tile([128, 2, 512], F32, tag="st", name="kvq_ps")
            for c in (2 * qi, 2 * qi + 1):
                nc.tensor.matmul(
                    ps[:, 0, :], wkvT_sb[:, c, :], x_tiles[b][:, c, :],
                    start=(c == 2 * qi), stop=(c == 2 * qi + 1),
                )
            if qi == 0:
                acc = apool.tile([128, 512], F32, tag="acc", name="kvacc")
                nc.vector.tensor_copy(acc, ps[:, 0, :])
                acc_kv[b] = acc
            elif qi < 3:
                tmp = apool.tile([128, 512], F32, tag="tmp", bufs=4, name="kvtmp")
                nc.vector.tensor_copy(tmp, ps[:, 0, :])
                merge_eng.tensor_tensor(
                    out=acc_kv[b], in0=tmp, in1=acc_kv[b], op=ADD
                )
            else:
                tmp = apool.tile([128, 512], F32, tag="tmp", bufs=4, name="kvtmp")
                nc.vector.tensor_copy(tmp, ps[:, 0, :])
                merge_eng.tensor_tensor(
                    out=kvT_sb[:, b * 512 : (b + 1) * 512],
                    in0=tmp, in1=acc_kv.pop(b), op=ADD,
                )

        def q_quarter(b, qi):
            """2-chunk q accumulation for query block b (rows 0:H only)."""
            ps = psP.tile([128, 2, 512], F32, tag="st", name="qq_ps")
            for c in (2 * qi, 2 * qi + 1):
                nc.tensor.matmul(
                    ps[0:H, 0, :], wqT_sb[:, c, :], x_tiles[b][:, c, :],
                    start=(c == 2 * qi), stop=(c == 2 * qi + 1),
                )
            if qi == 0:
                acc = apool.tile([128, 512], F32, tag="qac", name="qacc")
                nc.vector.tensor_copy(acc[0:H], ps[0:H, 0, :])
                acc_q[b] = acc
            elif qi < 3:
                nc.vector.tensor_tensor(
                    out=acc_q[b][0:H], in0=ps[0:H, 0, :],
                    in1=acc_q[b][0:H], op=ADD,
                )
            else:
                nc.vector.tensor_tensor(
                    out=qT_sb[0:H, b * 512 : (b + 1) * 512],
                    in0=ps[0:H, 0, :], in1=acc_q.pop(b)[0:H], op=ADD,
                )

        def vp_block(b):
            """V' rows for one t-block (4 s-chunks of transposes)."""
            vt_ps = psP.tile([128, 4, H], BF16, tag="st", name="vt_ps")
            for jj in range(4):
                s0 = b * 512 + jj * 128
                nc.tensor.transpose(
                    vt_ps[:, jj, :],
                    kvT_sb[H : 2 * H, s0 : s0 + 128],
                    ident_bf[H : 2 * H, H : 2 * H],
                )
            nc.vector.tensor_copy(
                Vp_sb[:, b * 4 : (b + 1) * 4, 0:H], vt_ps
            )

        # ---- step order: A0..A15, (A16,B0)..(A31,B15), B16..B31 ----
        order = [(0, j) for j in range(16)]
        for j in range(16, NS):
            order.append((0, j))
            order.append((1, j - 16))
        order += [(1, j) for j in range(16, NS)]
        assert len(order) == 2 * NS

        # weave schedule: step -> list of (kind, block, arg)
        # kv1,kv2 + q merges on DVE (free early); kv3..kv7 on GpSimd.
        weave = {}

        def put(g, *ops):
            weave.setdefault(g, []).extend(ops)

        put(0, ("kv", 1, 0), ("kv", 1, 1))
        put(1, ("kv", 1, 2), ("kv", 1, 3))
        put(2, ("kv", 2, 0), ("kv", 2, 1), ("xl", 5, None))
        put(3, ("kv", 2, 2), ("kv", 2, 3))
        put(4, ("kv", 3, 0), ("kv", 3, 1), ("vp", 0, None))
        put(5, ("kv", 3, 2), ("kv", 3, 3))
        put(6, ("kv", 4, 0), ("kv", 4, 1), ("vp", 1, None))
        put(7, ("kv", 4, 2), ("kv", 4, 3))
        put(8, ("q", 2, 0), ("q", 2, 1), ("xl", 6, None))
        put(9, ("q", 2, 2), ("q", 2, 3), ("vp", 2, None))
        put(10, ("q", 3, 0), ("q", 3, 1))
        put(11, ("q", 3, 2), ("q", 3, 3), ("vp", 3, None))
        put(13, ("vp", 4, None))
        put(14, ("kv", 5, 0))
        put(15, ("kv", 5, 1))
        put(16, ("xl", 7, None))
        put(17, ("kv", 5, 2))
        put(19, ("kv", 5, 3))
        put(21, ("kv", 6, 0))
        put(23, ("kv", 6, 1))
        put(25, ("kv", 6, 2))
        put(27, ("kv", 6, 3))
        put(29, ("kv", 7, 0))
        put(31, ("kv", 7, 1))
        put(33, ("kv", 7, 2))
        put(35, ("kv", 7, 3))
        put(39, ("vp", 5, None))
        put(41, ("vp", 6, None))
        put(43, ("vp", 7, None))

        pv_ps = psV.tile([128, 4, 512], F32, tag="pv")
        pt_tiles = [{}, {}]

        def pv_step(si, j, ts_list=(0, 1), pop=True):
            ptt = pt_tiles[si].pop(j) if pop else pt_tiles[si][j]
            for ts in ts_list:
                nc.tensor.matmul(
                    pv_ps[:, 2 * si + ts, :],
                    Vp_sb[:, j, :],
                    ptt[:, ts * 512 : (ts + 1) * 512],
                    start=(j == 0),
                    stop=(j == NS - 1),
                )

        oA_sb = opool.tile([H + 1, 1024], BF16, tag="oA")
        oB_sb = opool.tile([H + 1, 1024], BF16, tag="oB")

        for g, (si, j) in enumerate(order):
            for kind, b, arg in weave.get(g, []):
                if kind == "kv":
                    kv_quarter(b, arg, nc.vector if b <= 2 else nc.gpsimd)
                elif kind == "q":
                    q_quarter(b, arg)
                elif kind == "xl":
                    x_late(b)
                else:
                    vp_block(b)
            # A-stream PV drains + epilogue-A woven into the B-only tail
            if 48 <= g < 60:
                pv_step(0, g - 28)
            elif g == 60:
                nc.vector.tensor_copy(
                    oA_sb,
                    pv_ps[0 : H + 1, 0:2, :].rearrange("p a b -> p (a b)"),
                )
            elif g == 61:
                nc.sync.dma_start(out=out_ext[:, 0:1024], in_=oA_sb)
            # this step's mask half (each stream fetches its own cols)
            m_sb = mpool.tile([128, 1024], BF16, tag=("mA", "mB")[si])
            nc.sync.dma_start(
                out=m_sb,
                in_=maskT_ext[j * 128 : (j + 1) * 128,
                              si * 1024 : (si + 1) * 1024],
            )
            st_ps = psP.tile([128, 2, 512], F32, tag="st", name="st_ps")
            base = si * 1024
            for ts in range(2):
                t0 = base + ts * 512
                nc.tensor.matmul(
                    st_ps[:, ts, :],
                    kvT_sb[:, j * 128 : (j + 1) * 128],
                    qT_sb[:, t0 : t0 + 512],
                )
            ptt = ptpool.tile([128, 1024], BF16, tag="pt", name="ptt")
            nc.scalar.activation(
                ptt, st_ps.rearrange("p a b -> p (a b)"), Exp, scale=scale
            )
            nc.vector.tensor_mul(ptt, ptt, m_sb)
            pt_tiles[si][j] = ptt
            # PV lags: A by 12 (drains woven above), B by 3 (tightened tail)
            if si == 0 and j >= LAG_A:
                pv_step(0, j - LAG_A)
            elif si == 1:
                if j == NS - 2:
                    pv_step(1, j - LAG_B)
                    pv_step(1, j - LAG_B + 1)
                elif j == NS - 1:
                    pv_step(1, j - 2)
                    pv_step(1, j - 1)
                elif j >= LAG_B:
                    pv_step(1, j - LAG_B)

        # ---- tail: final B chunk split per supertile, copies off-exp ----
        pv_step(1, NS - 1, ts_list=(0,), pop=False)
        nc.scalar.copy(oB_sb[:, 0:512], pv_ps[0 : H + 1, 2, :])
        nc.sync.dma_start(out=out_ext[:, 1024:1536], in_=oB_sb[:, 0:512])
        pv_step(1, NS - 1, ts_list=(1,))
        nc.scalar.copy(oB_sb[:, 512:1024], pv_ps[0 : H + 1, 3, :])
        nc.sync.dma_start(out=out_ext[:, 1536:2048], in_=oB_sb[:, 512:1024])
    nc.compile()
    return nc


_NC_CACHE = {}


def _get_nc(shape_key):
    if shape_key not in _NC_CACHE:
        T_, D_, H_, Tq_ = shape_key
        _NC_CACHE[shape_key] = build_attention_core(T=T_, D=D_, H=H_, Tq=Tq_)
    return _NC_CACHE[shape_key]


def _pack_dchunks(wt):
    """[D, F] -> [128, DC*F]: partition-major packing of d-chunks."""
    Dv, Fv = wt.shape
    dc = Dv // 128
    return np.ascontiguousarray(
        wt.reshape(dc, 128, Fv).transpose(1, 0, 2).reshape(128, dc * Fv)
    )


def _prep_inputs(x, Wq, Wk, Wv, mask):
    """Host-side shard + transpose + cast + pack. Core c -> (batch c//2,
    half c%2). The x rows of the core's query half come first; mask columns
    get the same permutation so key order matches the permuted x rows."""
    x = np.ascontiguousarray(x, dtype=np.float32)
    mask = np.ascontiguousarray(mask, dtype=np.int32)
    Bv, Tv, Dv = x.shape
    Tq = Tv // 2
    ntb = Tv // 512
    dc = Dv // 128

    wqT = _pack_dchunks(
        np.ascontiguousarray(np.asarray(Wq, dtype=np.float32).T).astype(
            BF16NP
        )
    )
    wkvT = _pack_dchunks(
        np.concatenate(
            [np.asarray(Wk, np.float32).T, np.asarray(Wv, np.float32).T],
            axis=1,
        ).astype(BF16NP)
    )

    def block_xt(xb):
        # [T, D] -> [ (tb, 128part), (d-chunk, 512) ]
        xt = xb.T.astype(BF16NP)  # [D, T]
        x4 = xt.reshape(dc, 128, ntb, 512).transpose(2, 1, 0, 3)
        return np.ascontiguousarray(x4.reshape(ntb * 128, dc * 512))

    # mask is shared across batches: only two variants (one per half)
    m0 = mask[0, 0:Tq, :]  # [t, s] for half 0
    m1 = np.concatenate([mask[0, Tq:, Tq:], mask[0, Tq:, :Tq]], axis=1)
    maskT0 = np.ascontiguousarray(m0.T.astype(BF16NP))
    maskT1 = np.ascontiguousarray(m1.T.astype(BF16NP))

    in_maps = []
    for c in range(NCORES):
        b, half = c // 2, c % 2
        if half == 0:
            xc = x[b]
            mT = maskT0
        else:
            xc = np.concatenate([x[b, Tq:], x[b, :Tq]], axis=0)
            mT = maskT1
        in_maps.append(
            {
                "xt": block_xt(xc),
                "wqt": wqT,
                "wkvt": wkvT,
                "maskt": mT,
            }
        )
    return in_maps


def kernel(x, Wq, Wk, Wv, mask, _trace=False):
    x = np.asarray(x)
    Bv, Tv, Dv = x.shape
    Hv = np.asarray(Wq).shape[0]
    Tq = Tv // 2
    nc = _get_nc((Tv, Dv, Hv, Tq))
    in_maps = _prep_inputs(
        np.asarray(x), np.asarray(Wq), np.asarray(Wk), np.asarray(Wv),
        np.asarray(mask),
    )
    res = run_bass_kernel_spmd(
        nc, in_maps, core_ids=list(range(NCORES)), trace=_trace
    )
    out = np.empty((Bv, Tv, Hv), dtype=np.float32)
    for c in range(NCORES):
        b, half = c // 2, c % 2
        r = np.asarray(res.results[c]["out"], dtype=np.float32)
        out[b, half * Tq : (half + 1) * Tq] = (r[0:Hv] / r[Hv : Hv + 1]).T
    if _trace:
        kernel.last_results = res
    return out


# revision 10
# speedup vs baseline: 1.1635x; 1.1635x over previous
"""Single-head attention (B=4, T=4096, D=1024, H=64, fp32 in/out) on 8 TRN2
NeuronCores.

Sharding: one core per (batch, T-half) pair -> 8 shards, no collectives.
Host pre-transposes/pre-casts/pre-packs inputs (zero device-side transposes):
  xt      [8*128, 8*512] bf16  per t-block: [128 part, (d-chunk, 512)] of
                               x[b]^T (query t-blocks first)
  wqt     [128, 8*64]    bf16  Wq^T packed [part, (d-chunk, 64)]
  wkvt    [128, 8*128]   bf16  [Wk^T | Wv^T] packed likewise
  maskt   [4096, 2048]   bf16  mask slice transposed to [s, t]
Each core returns un-normalized [65, 2048] (out'^T rows 0:64, softmax
denominator row 64); the host does the final divide + transpose.

Architecture of the schedule. Act's exp stream is the hard floor (8.4M
elements / 128 lanes / ~0.9 GHz ~= 72 us) and the PE is co-saturated
(ST 128 + PV 128 + proj 96 matmuls at ~215 ns N=512 issue ~= 76 us), so
the kernel must (a) start the exp stream early, (b) never let any
engine's in-order stream stall it, (c) smooth all projection work into
fine grains that fit per-step slack:

  - Two column streams: A = query cols 0:1024 (PV supertiles 0,1),
    B = cols 1024:2048 (2,3). Order A0..A15, (A16,B0)..(A31,B15),
    B16..B31. exp-A0 needs only q0,q1,kv0.
  - Stream A's PV lags 12 steps: steps 0..11 carry no PV matmuls, which
    is exactly the slack that absorbs the kv1-kv4 + q2,q3 projection
    quarters (2 matmuls each, PSUM-gen-local). Stream B's PV lags 3.
    A's last 12 PV chunks drain into the B-only tail steps.
  - Projection quarters accumulate 2 d-chunks in PSUM, a DVE copy moves
    them to SBUF f32, and an add chain (DVE for early blocks, GpSimd --
    which has no other mid-kernel work -- for late blocks) merges into
    bf16 kvT. No PSUM generation outlives its step, so the 2-deep ST
    ping-pong never serializes.
  - DMA: queues are FIFO internally but round-robin against each other,
    so the sync HW queue carries x0,x1 chunk-pipelined (projections
    chase landing chunks), then x2,x3,x4, then per-step mask halves
    (each stream fetches only its own [128,1024] half when it needs
    it); x5-x7 dispatch from inside the loop. Weights ride the scalar
    HW queue in parallel. Deep PT ring (20) + lagged PV decouple mask
    lateness from the exp stream.
  - ST matmuls use full-128-row kvT / zero-padded qT so the whole PE
    array stays active (HAM clock gate 1.2 -> 2.4 GHz); dummy warmup
    matmuls on a memset tile ramp the clock with no DMA dependency.
  - Tail: stream B's PV trail tightens over the last two steps, the
    final PV/copy/store is split per 512-col supertile, epilogue copies
    run on Vector/Act after the last exp; host divides.
"""

import sys

if "/opt/trn_rl_repo" not in sys.path:
    sys.path.insert(0, "/opt/trn_rl_repo")

from contextlib import ExitStack

import numpy as np
import ml_dtypes

import concourse.bass as bass
import concourse.tile as tile
from concourse import bacc, mybir
from concourse.bass_utils import run_bass_kernel_spmd
from concourse.masks import make_identity

F32 = mybir.dt.float32
BF16 = mybir.dt.bfloat16

B, T, D, H = 4, 4096, 1024, 64
NCORES = 8
TQ = T // 2  # query rows per core

BF16NP = ml_dtypes.bfloat16


def build_attention_core(T=T, D=D, H=H, Tq=TQ):
    """Build the per-core Bass graph. Every core runs the same graph."""
    assert D % 128 == 0 and T % 1024 == 0 and Tq % 1024 == 0 and H == 64
    DC = D // 128          # d chunks (8)
    NS = T // 128          # s chunks (32)
    NTB = T // 512         # x t-blocks (8)
    NQB = Tq // 512        # query t-blocks (4)
    LAG_A = 12             # stream A PV lag (frees early steps of PE)
    LAG_B = 3              # stream B PV lag
    scale = 1.0 / float(np.sqrt(D))
    Exp = mybir.ActivationFunctionType.Exp
    ADD = mybir.AluOpType.add

    nc = bacc.Bacc("TRN2", target_bir_lowering=False, debug=False,
                   num_devices=NCORES)
    xT_ext = nc.declare_dram_parameter("xt", [NTB * 128, DC * 512], BF16,
                                       isOutput=False)
    wqT_ext = nc.declare_dram_parameter("wqt", [128, DC * H], BF16,
                                        isOutput=False)
    wkvT_ext = nc.declare_dram_parameter("wkvt", [128, DC * 2 * H], BF16,
                                         isOutput=False)
    maskT_ext = nc.declare_dram_parameter("maskt", [T, Tq], BF16,
                                          isOutput=False)
    out_ext = nc.declare_dram_parameter("out", [H + 1, Tq], BF16,
                                        isOutput=True)

    with tile.TileContext(nc) as tc, ExitStack() as ctx:
        singles = ctx.enter_context(tc.tile_pool(name="singles", bufs=1))
        xin = ctx.enter_context(tc.tile_pool(name="xin", bufs=1))
        mpool = ctx.enter_context(tc.tile_pool(name="mpool", bufs=8))
        ptpool = ctx.enter_context(tc.tile_pool(name="ptpool", bufs=24))
        apool = ctx.enter_context(tc.tile_pool(name="apool", bufs=2))
        opool = ctx.enter_context(tc.tile_pool(name="opool", bufs=1))
        # PSUM: tag "st" [128,2,512] f32 x2 (4 banks) shared by ST tiles,
        # projection quarters, V' transposes and warmup; tag "pv"
        # [128,4,512] f32 x1 (4 banks) = (stream,ts) accumulators.
        psP = ctx.enter_context(tc.tile_pool(name="psP", bufs=2,
                                             space="PSUM"))
        psV = ctx.enter_context(tc.tile_pool(name="psV", bufs=1,
                                             space="PSUM"))

        # ---- persistent SBUF ----
        wqT_sb = singles.tile([128, DC, H], BF16)
        wkvT_sb = singles.tile([128, DC, 2 * H], BF16)
        ident_bf = singles.tile([128, 128], BF16)
        warm_sb = singles.tile([128, 512], BF16)
        kvT_sb = singles.tile([128, T], BF16)   # rows 0:64 kT, 64:128 vT
        qT_sb = singles.tile([128, Tq], BF16)   # rows 64:128 zero
        Vp_sb = singles.tile([128, NS, 128], BF16)  # V' = [V | 1 | 0pad]

        # ---- weights on the scalar HW queue (parallel with x on sync) ----
        nc.scalar.dma_start(
            out=wqT_sb.rearrange("p a b -> p (a b)"), in_=wqT_ext[:, :]
        )
        nc.scalar.dma_start(
            out=wkvT_sb.rearrange("p a b -> p (a b)"), in_=wkvT_ext[:, :]
        )

        # ---- x0, x1 chunk-pipelined, then x2-x4, on the sync HW queue ----
        x_tiles = {}
        for b in range(2):
            x_sb = xin.tile([128, DC, 512], BF16, tag="x", bufs=NTB,
                            name="x_sb")
            for c in range(0, DC, 2):
                nc.sync.dma_start(
                    out=x_sb[:, c : c + 2, :].rearrange("p a b -> p (a b)"),
                    in_=xT_ext[b * 128 : (b + 1) * 128,
                               c * 512 : (c + 2) * 512],
                )
            x_tiles[b] = x_sb
        for b in range(2, 5):
            x_sb = xin.tile([128, DC, 512], BF16, tag="x", bufs=NTB,
                            name="x2_sb")
            nc.sync.dma_start(
                out=x_sb.rearrange("p a b -> p (a b)"),
                in_=xT_ext[b * 128 : (b + 1) * 128, :],
            )
            x_tiles[b] = x_sb

        # ---- step order: A0..A15, (A16,B0)..(A31,B15), B16..B31 ----
        order = [(0, j) for j in range(16)]
        for j in range(16, NS):
            order.append((0, j))
            order.append((1, j - 16))
        order += [(1, j) for j in range(16, NS)]
        assert len(order) == 2 * NS

        # masks are prefetched 6 steps ahead of their mul so a late mask
        # can never stall DVE's in-order stream (which also carries the
        # PSUM-freeing projection copies)
        m_tiles = {}

        def fetch_mask(h):
            si_h, j_h = order[h]
            m_sb = mpool.tile([128, 1024], BF16, tag=("mA", "mB")[si_h],
                              name="m_sb")
            nc.sync.dma_start(
                out=m_sb,
                in_=maskT_ext[j_h * 128 : (j_h + 1) * 128,
                              si_h * 1024 : (si_h + 1) * 1024],
            )
            m_tiles[h] = m_sb

        for h in range(6):
            fetch_mask(h)

        def x_late(b):
            """Dispatch a late x block from inside the loop (sync queue)."""
            x_sb = xin.tile([128, DC, 512], BF16, tag="x", bufs=NTB,
                            name="xl_sb")
            nc.sync.dma_start(
                out=x_sb.rearrange("p a b -> p (a b)"),
                in_=xT_ext[b * 128 : (b + 1) * 128, :],
            )
            x_tiles[b] = x_sb

        # ---- gpsimd setup (runs while DMAs fly) ----
        nc.gpsimd.memset(warm_sb, 1.0)
        nc.gpsimd.memset(qT_sb[H : 2 * H, :], 0.0)
        make_identity(nc, ident_bf)
        nc.gpsimd.memset(Vp_sb[:, :, H + 1 : 128], 0.0)
        nc.gpsimd.memset(Vp_sb[:, :, H : H + 1], 1.0)

        # ---- PE warmup: ramp the clock before real work (no DMA dep) ----
        for _ in range(3):
            w_ps = psP.tile([128, 2, 512], F32, tag="st", name="w_ps")
            for i in range(2):
                nc.tensor.matmul(w_ps[:, i, :], warm_sb[:, 0:128], warm_sb)

        # ---- prologue: q0 + kv0 interleaved (chase x0), then q1 ----
        qkv_ps = psP.tile([128, 2, 512], F32, tag="st", name="qkv_ps")
        for c in range(DC):
            nc.tensor.matmul(
                qkv_ps[0:H, 0, :], wqT_sb[:, c, :], x_tiles[0][:, c, :],
                start=(c == 0), stop=(c == DC - 1),
            )
            nc.tensor.matmul(
                qkv_ps[:, 1, :], wkvT_sb[:, c, :], x_tiles[0][:, c, :],
                start=(c == 0), stop=(c == DC - 1),
            )
        nc.vector.tensor_copy(qT_sb[0:H, 0:512], qkv_ps[0:H, 0, :])
        nc.vector.tensor_copy(kvT_sb[:, 0:512], qkv_ps[:, 1, :])
        q1_ps = psP.tile([128, 2, 512], F32, tag="st", name="q1_ps")
        for c in range(DC):
            nc.tensor.matmul(
                q1_ps[0:H, 0, :], wqT_sb[:, c, :], x_tiles[1][:, c, :],
                start=(c == 0), stop=(c == DC - 1),
            )
        nc.vector.tensor_copy(qT_sb[0:H, 512:1024], q1_ps[0:H, 0, :])

        # ---- projection quarters (2 d-chunks in PSUM, merged via SBUF) ----
        acc_kv = {}
        acc_q = {}

        def kv_quarter(b, qi, merge_eng):
            """2-chunk kv accumulation for block b; merge via SBUF f32."""
            ps = psP.tile([128, 2, 512], F32, tag="st", name="kvq_ps")
            for c in (2 * qi, 2 * qi + 1):
                nc.tensor.matmul(
                    ps[:, 0, :], wkvT_sb[:, c, :], x_tiles[b][:, c, :],
                    start=(c == 2 * qi), stop=(c == 2 * qi + 1),
                )
            if qi == 0:
                acc = apool.tile([128, 512], F32, tag="acc", name="kvacc")
                nc.vector.tensor_copy(acc, ps[:, 0, :])
                acc_kv[b] = acc
            elif qi < 3:
                tmp = apool.tile([128, 512], F32, tag="tmp", bufs=4, name="kvtmp")
                nc.vector.tensor_copy(tmp, ps[:, 0, :])
                merge_eng.tensor_tensor(
                    out=acc_kv[b], in0=tmp, in1=acc_kv[b], op=ADD
                )
            else:
                tmp = apool.tile([128, 512], F32, tag="tmp", bufs=4, name="kvtmp")
                nc.vector.tensor_copy(tmp, ps[:, 0, :])
                merge_eng.tensor_tensor(
                    out=kvT_sb[:, b * 512 : (b + 1) * 512],
                    in0=tmp, in1=acc_kv.pop(b), op=ADD,
                )

        def q_quarter(b, qi):
            """2-chunk q accumulation for query block b (rows 0:H only)."""
            ps = psP.tile([128, 2, 512], F32, tag="st", name="qq_ps")
            for c in (2 * qi, 2 * qi + 1):
                nc.tensor.matmul(
                    ps[0:H, 0, :], wqT_sb[:, c, :], x_tiles[b][:, c, :],
                    start=(c == 2 * qi), stop=(c == 2 * qi + 1),
                )
            if qi == 0:
                acc = apool.tile([128, 512], F32, tag="qac", name="qacc")
                nc.vector.tensor_copy(acc[0:H], ps[0:H, 0, :])
                acc_q[b] = acc
            elif qi < 3:
                nc.vector.tensor_tensor(
                    out=acc_q[b][0:H], in0=ps[0:H, 0, :],
                    in1=acc_q[b][0:H], op=ADD,
                )
            else:
                nc.vector.tensor_tensor(
                    out=qT_sb[0:H, b * 512 : (b + 1) * 512],
                    in0=ps[0:H, 0, :], in1=acc_q.pop(b)[0:H], op=ADD,
                )

        def vp_block(b):
            """V' rows for one t-block (4 s-chunks of transposes)."""
            vt_ps = psP.tile([128, 4, H], BF16, tag="st", name="vt_ps")
            for jj in range(4):
                s0 = b * 512 + jj * 128
                nc.tensor.transpose(
                    vt_ps[:, jj, :],
                    kvT_sb[H : 2 * H, s0 : s0 + 128],
                    ident_bf[H : 2 * H, H : 2 * H],
                )
            nc.vector.tensor_copy(
                Vp_sb[:, b * 4 : (b + 1) * 4, 0:H], vt_ps
            )


        # weave schedule: step -> list of (kind, block, arg)
        # kv1,kv2 + q merges on DVE (free early); kv3..kv7 on GpSimd.
        weave = {}

        def put(g, *ops):
            weave.setdefault(g, []).extend(ops)

        put(0, ("kv", 1, 0), ("kv", 1, 1), ("xl", 5, None))
        put(1, ("kv", 1, 2), ("kv", 1, 3))
        put(2, ("q", 2, 0), ("q", 2, 1), ("vp", 0, None))
        put(3, ("q", 2, 2), ("q", 2, 3), ("vp", 1, None))
        put(4, ("kv", 2, 0), ("kv", 2, 1), ("xl", 6, None))
        put(5, ("kv", 2, 2), ("kv", 2, 3))
        put(6, ("q", 3, 0), ("q", 3, 1))
        put(7, ("q", 3, 2), ("q", 3, 3), ("vp", 2, None))
        put(8, ("kv", 3, 0), ("kv", 3, 1))
        put(9, ("kv", 3, 2), ("kv", 3, 3))
        put(10, ("kv", 4, 0), ("kv", 4, 1))
        put(11, ("kv", 4, 2), ("kv", 4, 3), ("vp", 3, None))
        put(12, ("xl", 7, None))
        put(13, ("kv", 5, 0), ("vp", 4, None))
        put(14, ("kv", 5, 1))
        put(15, ("kv", 5, 2))
        put(17, ("kv", 5, 3))
        put(19, ("kv", 6, 0))
        put(21, ("kv", 6, 1))
        put(23, ("kv", 6, 2))
        put(25, ("kv", 6, 3))
        put(27, ("kv", 7, 0))
        put(29, ("kv", 7, 1))
        put(31, ("kv", 7, 2))
        put(33, ("kv", 7, 3))
        put(35, ("vp", 5, None))
        put(37, ("vp", 6, None))
        put(39, ("vp", 7, None))

        pv_ps = psV.tile([128, 4, 512], F32, tag="pv")
        pt_tiles = [{}, {}]

        def pv_step(si, j, ts_list=(0, 1), pop=True):
            ptt = pt_tiles[si].pop(j) if pop else pt_tiles[si][j]
            for ts in ts_list:
                nc.tensor.matmul(
                    pv_ps[:, 2 * si + ts, :],
                    Vp_sb[:, j, :],
                    ptt[:, ts * 512 : (ts + 1) * 512],
                    start=(j == 0),
                    stop=(j == NS - 1),
                )

        oA_sb = opool.tile([H + 1, 1024], BF16, tag="oA")
        oB_sb = opool.tile([H + 1, 1024], BF16, tag="oB")

        for g, (si, j) in enumerate(order):
            for kind, b, arg in weave.get(g, []):
                if kind == "kv":
                    kv_quarter(b, arg, nc.vector if b <= 2 else nc.gpsimd)
                elif kind == "q":
                    q_quarter(b, arg)
                elif kind == "xl":
                    x_late(b)
                else:
                    vp_block(b)
            # A-stream PV drains + epilogue-A woven into the B-only tail
            if 48 <= g < 60:
                pv_step(0, g - 28)
            elif g == 60:
                nc.vector.tensor_copy(
                    oA_sb,
                    pv_ps[0 : H + 1, 0:2, :].rearrange("p a b -> p (a b)"),
                )
            elif g == 61:
                nc.sync.dma_start(out=out_ext[:, 0:1024], in_=oA_sb)
            if g + 6 < 2 * NS:
                fetch_mask(g + 6)
            st_ps = psP.tile([128, 2, 512], F32, tag="st", name="st_ps")
            base = si * 1024
            for ts in range(2):
                t0 = base + ts * 512
                nc.tensor.matmul(
                    st_ps[:, ts, :],
                    kvT_sb[:, j * 128 : (j + 1) * 128],
                    qT_sb[:, t0 : t0 + 512],
                )
            ptt = ptpool.tile([128, 1024], BF16, tag="pt", name="ptt")
            nc.scalar.activation(
                ptt, st_ps.rearrange("p a b -> p (a b)"), Exp, scale=scale
            )
            nc.vector.tensor_mul(ptt, ptt, m_tiles[g])
            pt_tiles[si][j] = ptt
            # PV lags: A by 12 (drains woven above), B by 3 (tightened tail)
            if si == 0 and j >= LAG_A:
                pv_step(0, j - LAG_A)
            elif si == 1:
                if j == NS - 2:
                    pv_step(1, j - LAG_B)
                    pv_step(1, j - LAG_B + 1)
                elif j == NS - 1:
                    pv_step(1, j - 2)
                    pv_step(1, j - 1)
                elif j >= LAG_B:
                    pv_step(1, j - LAG_B)

        # ---- tail: final B chunk split per supertile, copies off-exp ----
        pv_step(1, NS - 1, ts_list=(0,), pop=False)
        nc.scalar.copy(oB_sb[:, 0:512], pv_ps[0 : H + 1, 2, :])
        nc.sync.dma_start(out=out_ext[:, 1024:1536], in_=oB_sb[:, 0:512])
        pv_step(1, NS - 1, ts_list=(1,))
        nc.scalar.copy(oB_sb[:, 512:1024], pv_ps[0 : H + 1, 3, :])
        nc.sync.dma_start(out=out_ext[:, 1536:2048], in_=oB_sb[:, 512:1024])
    nc.compile()
    return nc


_NC_CACHE = {}


def _get_nc(shape_key):
    if shape_key not in _NC_CACHE:
        T_, D_, H_, Tq_ = shape_key
        _NC_CACHE[shape_key] = build_attention_core(T=T_, D=D_, H=H_, Tq=Tq_)
    return _NC_CACHE[shape_key]


def _pack_dchunks(wt):
    """[D, F] -> [128, DC*F]: partition-major packing of d-chunks."""
    Dv, Fv = wt.shape
    dc = Dv // 128
    return np.ascontiguousarray(
        wt.reshape(dc, 128, Fv).transpose(1, 0, 2).reshape(128, dc * Fv)
    )


def _prep_inputs(x, Wq, Wk, Wv, mask):
    """Host-side shard + transpose + cast + pack. Core c -> (batch c//2,
    half c%2). The x rows of the core's query half come first; mask columns
    get the same permutation so key order matches the permuted x rows."""
    x = np.ascontiguousarray(x, dtype=np.float32)
    mask = np.ascontiguousarray(mask, dtype=np.int32)
    Bv, Tv, Dv = x.shape
    Tq = Tv // 2
    ntb = Tv // 512
    dc = Dv // 128

    wqT = _pack_dchunks(
        np.ascontiguousarray(np.asarray(Wq, dtype=np.float32).T).astype(
            BF16NP
        )
    )
    wkvT = _pack_dchunks(
        np.concatenate(
            [np.asarray(Wk, np.float32).T, np.asarray(Wv, np.float32).T],
            axis=1,
        ).astype(BF16NP)
    )

    def block_xt(xb):
        # [T, D] -> [ (tb, 128part), (d-chunk, 512) ]
        xt = xb.T.astype(BF16NP)  # [D, T]
        x4 = xt.reshape(dc, 128, ntb, 512).transpose(2, 1, 0, 3)
        return np.ascontiguousarray(x4.reshape(ntb * 128, dc * 512))

    # mask is shared across batches: only two variants (one per half)
    m0 = mask[0, 0:Tq, :]  # [t, s] for half 0
    m1 = np.concatenate([mask[0, Tq:, Tq:], mask[0, Tq:, :Tq]], axis=1)
    maskT0 = np.ascontiguousarray(m0.T.astype(BF16NP))
    maskT1 = np.ascontiguousarray(m1.T.astype(BF16NP))

    in_maps = []
    for c in range(NCORES):
        b, half = c // 2, c % 2
        if half == 0:
            xc = x[b]
            mT = maskT0
        else:
            xc = np.concatenate([x[b, Tq:], x[b, :Tq]], axis=0)
            mT = maskT1
        in_maps.append(
            {
                "xt": block_xt(xc),
                "wqt": wqT,
                "wkvt": wkvT,
                "maskt": mT,
            }
        )
    return in_maps


def kernel(x, Wq, Wk, Wv, mask, _trace=False):
    x = np.asarray(x)
    Bv, Tv, Dv = x.shape
    Hv = np.asarray(Wq).shape[0]
    Tq = Tv // 2
    nc = _get_nc((Tv, Dv, Hv, Tq))
    in_maps = _prep_inputs(
        np.asarray(x), np.asarray(Wq), np.asarray(Wk), np.asarray(Wv),
        np.asarray(mask),
    )
    res = run_bass_kernel_spmd(
        nc, in_maps, core_ids=list(range(NCORES)), trace=_trace
    )
    out = np.empty((Bv, Tv, Hv), dtype=np.float32)
    for c in range(NCORES):
        b, half = c // 2, c % 2
        r = np.asarray(res.results[c]["out"], dtype=np.float32)
        out[b, half * Tq : (half + 1) * Tq] = (r[0:Hv] / r[Hv : Hv + 1]).T
    if _trace:
        kernel.last_results = res
    return out
